# revision 5
# baseline (speedup 1.0000x reference)
"""Trainium2 Bass kernel for nn_DepthPredictorMultiView.

Self-contained: takes FULL inputs (as produced by the reference's
setup_inputs), shards per scene across 8 NeuronCores, runs a Bass/Tile
kernel per core, gathers the full outputs.

Per-core device pipeline (scene-parallel):
  - DLT triangulation per keypoint: M = A^T A (4x4 symmetric, built from a
    host-precomputed 10x7 quadratic-form matrix), smallest eigenvalue via
    Newton on the characteristic quartic, eigenvector via 4D cross products.
  - z per view from the inverse-extrinsics row, validity, near/far
    reductions.
  - Disparity fusion: maps stream through SBUF; fused values are placed
    into dense per-chunk VAL tiles with GPSIMD local_scatter (f32 values
    scattered as interleaved u16 pairs; cross-partition routing for view 1
    via PE transposes), then merged elementwise and written out.
"""

import itertools
import numpy as np

import concourse.bass as bass
import concourse.bacc as bacc
import concourse.mybir as mybir
import concourse.dve_ops as dve_ops
from concourse.dve_ops import DveOp, has_src1
from concourse.dve_spec import Spec, Src0, Src1, C0, C1, Zero, One, select, sq, lower
from concourse.dve_uop import DveOpSpec
from concourse.tile import TileContext
from concourse.masks import make_identity
from concourse import bass_isa

# ---------------- problem constants (hardcoded) ----------------
B, V, H, W = 8, 2, 1024, 1024
N = 131072
TH_MAX = 500.0
P = 128          # partitions
RB = 8           # row blocks per map (1024/128)
CH = 2           # column halves per row block
CW = 512         # chunk width in f32
FD = 176         # keypoint slots per partition
GW = 24          # padded per-(partition, chunk) group width
NG = RB * CH     # 16 groups per view
EW = NG * GW     # 384: expanded/compacted value layout width
SQ = 10          # transpose squares for view-1 routing
SQH = SQ // 2
NEWTON_ITERS = 9
TINY = 1e-30

F32 = mybir.dt.float32
I16 = mybir.dt.int16
U16 = mybir.dt.uint16

LAST_RESULTS = None     # BassKernelResults of the most recent run (for test.py)

# ---------------- custom DVE ops ----------------
_REGISTERED = {}


def _reg(name, body, reference):
    if name in _REGISTERED:
        return _REGISTERED[name]
    spec = Spec(body=body, reference=reference)
    op = DveOp(name, spec, subdim=False, uops_sha={})
    dve_ops.OPS.append(op)
    dve_ops.CUSTOM_DVE_SPECS[name] = spec
    row = max(dve_ops._SUB_OPCODE_FOR_NAME.values()) + 1
    assert row < 0x20
    dve_ops._SUB_OPCODE_FOR_NAME[name] = row
    for ver in ("v3", "v4"):
        s = DveOpSpec(
            name=name,
            opcode=row,
            uops=lower(spec, ver=ver),
            rd1_en=has_src1(spec),
        )
        op.uops_sha[ver] = s.sha(ver)
    _REGISTERED[name] = op
    return op


OP_SQSUM2 = _reg(
    "ANT_SQSUM2", sq(Src0) + sq(Src1),
    lambda in0, in1, s0, s1, imm2: (in0 * in0 + in1 * in1).astype(np.float32),
)
OP_MAD2 = _reg(
    "ANT_MAD2", Src0 * C0 + Src1 * C1,
    lambda in0, in1, s0, s1, imm2: (in0 * s0 + in1 * s1).astype(np.float32),
)
OP_MAD1A = _reg(
    "ANT_MAD1A", Src0 * C0 + Src1,
    lambda in0, in1, s0, s1, imm2: (in0 * s0 + in1).astype(np.float32),
)
OP_AD2C = _reg(
    "ANT_AD2C", Src0 + Src1 + C0,
    lambda in0, in1, s0, s1, imm2: (in0 + in1 + s0).astype(np.float32),
)
OP_SQSUB = _reg(
    "ANT_SQSUB", Src1 - sq(Src0),
    lambda in0, in1, s0, s1, imm2: (in1 - in0 * in0).astype(np.float32),
)
OP_LC = _reg(
    "ANT_LC", (Src0 - Src1) * Src0,
    lambda in0, in1, s0, s1, imm2: ((in0 - in1) * in0).astype(np.float32),
)
OP_RANGE = _reg(
    "ANT_RANGE", (Src0 > Zero) & (Src0 < C0),
    lambda in0, in1, s0, s1, imm2: ((in0 > 0) & (in0 < s0)).astype(np.float32),
)
OP_SELHALF = _reg(
    "ANT_SELHALF", select(Src1 > Zero, Src0 * C0, Zero),
    lambda in0, in1, s0, s1, imm2: np.where(in1 > 0, in0 * s0, 0.0).astype(np.float32),
)
OP_SELC = _reg(
    "ANT_SELC", select(Src1 > Zero, Src0, C0),
    lambda in0, in1, s0, s1, imm2: np.where(in1 > 0, in0, s0).astype(np.float32),
)
OP_MERGE = _reg(
    "ANT_MERGE", Src0 - Src0 * (Src1 > Zero) * C0 + Src1,
    lambda in0, in1, s0, s1, imm2: (in0 - in0 * (in1 > 0) * s0 + in1).astype(np.float32),
)
OP_MSUB = _reg(
    "ANT_MSUB", Src0 * Src1,
    lambda in0, in1, s0, s1, imm2: (in0 * in1).astype(np.float32),
)


# ---------------- builder ----------------

def _build_nc():
    nc = bacc.Bacc()

    kpt = nc.declare_dram_parameter("kpt", [P, 4 * FD], F32, isOutput=False)
    cst = nc.declare_dram_parameter("cst", [P, 96], F32, isOutput=False)
    maps = nc.declare_dram_parameter("maps", [V, H, W], F32, isOutput=False)
    eidx = nc.declare_dram_parameter("eidx", [P, 2 * FD], I16, isOutput=False)
    pidx = nc.declare_dram_parameter("pidx", [P, 4 * FD], I16, isOutput=False)
    cidx = nc.declare_dram_parameter("cidx", [P, 2 * SQ * P], I16, isOutput=False)
    sidx0 = nc.declare_dram_parameter("sidx0", [P, NG * 2 * GW], I16, isOutput=False)
    sidx1 = nc.declare_dram_parameter("sidx1", [P, NG * 2 * GW], I16, isOutput=False)

    out_maps = nc.declare_dram_parameter("out_maps", [V, H, W], F32, isOutput=True)
    stats = nc.declare_dram_parameter("stats", [1, 8], F32, isOutput=True)

    with TileContext(nc) as tc:
        with (
            tc.tile_pool(name="main", bufs=1) as pool,
            tc.tile_pool(name="psum", bufs=2, space="PSUM") as psp,
        ):
            _emit(nc, tc, pool, psp, kpt, cst, maps, eidx, pidx, cidx, sidx0,
                  sidx1, out_maps, stats)
    nc.finalize()
    return nc


def _emit(nc, tc, pool, psp, kpt, cst, maps, eidx, pidx, cidx, sidx0, sidx1,
          out_maps, stats):
    cnt = itertools.count()

    def newt(shape=(P, FD), dtype=F32, name=None):
        if name is None:
            return pool.tile(list(shape), dtype, name=f"w{next(cnt)}",
                             tag="work", bufs=64)
        return pool.tile(list(shape), dtype, name=name)

    _ARITH = {mybir.AluOpType.mult, mybir.AluOpType.add,
              mybir.AluOpType.subtract, mybir.AluOpType.max,
              mybir.AluOpType.min}

    def tt(a, b, op, out=None):
        o = out if out is not None else newt()
        eng = nc.any if op in _ARITH else nc.vector
        eng.tensor_tensor(out=o[:], in0=a[:], in1=b[:], op=op)
        return o

    def ts(a, s1, op0, s2=None, op1=None, out=None):
        o = out if out is not None else newt()
        eng = nc.any if op0 in _ARITH and (op1 is None or op1 in _ARITH) \
            else nc.vector
        eng.tensor_scalar(
            out=o[:], in0=a[:], scalar1=s1, scalar2=s2,
            op0=op0, op1=op1 if op1 is not None else mybir.AluOpType.bypass)
        return o

    def stt(a, s, b, op0, op1, out=None):
        o = out if out is not None else newt()
        nc.vector.scalar_tensor_tensor(
            out=o[:], in0=a[:], scalar=s, in1=b[:], op0=op0, op1=op1)
        return o

    def cdve(op, in0, in1=None, s0=0.0, s1=0.0, out=None, shape=(P, FD)):
        o = out if out is not None else newt(shape)
        nc.vector._custom_dve(
            op, out=o[:], in0=in0[:],
            in1=in1[:] if in1 is not None else None, s0=s0, s1=s1)
        return o

    def recip(a, out=None):
        o = out if out is not None else newt()
        nc.vector.reciprocal(out=o[:], in_=a[:])
        return o

    MUL = mybir.AluOpType.mult
    ADD = mybir.AluOpType.add
    SUB = mybir.AluOpType.subtract

    # ---------- DMA in ----------
    kp = newt((P, 4 * FD), name="kp")
    nc.sync.dma_start(out=kp[:], in_=kpt[:])
    cstt = newt((P, 96), name="cstt")
    nc.sync.dma_start(out=cstt[:], in_=cst[:])
    sc = lambda j: cstt[:, j:j + 1]  # noqa: E731

    eidx_t = newt((P, 2 * FD), I16, name="eidx_t")
    nc.sync.dma_start(out=eidx_t[:], in_=eidx[:])
    pidx_t = newt((P, 4 * FD), I16, name="pidx_t")
    nc.sync.dma_start(out=pidx_t[:], in_=pidx[:])
    cidx_t = newt((P, 2 * SQ * P), I16, name="cidx_t")
    nc.sync.dma_start(out=cidx_t[:], in_=cidx[:])
    sidx0_t = newt((P, NG * 2 * GW), I16, name="sidx0_t")
    nc.sync.dma_start(out=sidx0_t[:], in_=sidx0[:])
    sidx1_t = newt((P, NG * 2 * GW), I16, name="sidx1_t")
    nc.sync.dma_start(out=sidx1_t[:], in_=sidx1[:])

    mch = []  # map chunks (v, rb) -> (128, 1024) tile
    for v in range(V):
        row = []
        for rb in range(RB):
            t = newt((P, W), name=f"mch{v}_{rb}")
            nc.sync.dma_start(out=t[:], in_=maps[v, rb * P:(rb + 1) * P, :])
            row.append(t)
        mch.append(row)

    x0 = kp[:, 0 * FD:1 * FD]
    y0 = kp[:, 1 * FD:2 * FD]
    x1 = kp[:, 2 * FD:3 * FD]
    y1 = kp[:, 3 * FD:4 * FD]

    class A:  # AP wrapper so helpers can take raw slices uniformly
        def __init__(self, ap):
            self.ap = ap

        def __getitem__(self, k):
            return self.ap

    x0, y0, x1, y1 = A(x0), A(y0), A(x1), A(y1)

    # ---------- Phase A: features, trace, M ----------
    s0 = cdve(OP_SQSUM2, x0, y0)
    s1 = cdve(OP_SQSUM2, x1, y1)
    # trace = Kt . feats + Kt6  (Kt at consts cols 70..76)
    u1 = cdve(OP_MAD2, s0, x0, sc(70), sc(71))
    u2 = cdve(OP_MAD2, y0, s1, sc(72), sc(73))
    u3 = cdve(OP_MAD2, x1, y1, sc(74), sc(75))
    u12 = tt(u1, u2, ADD)
    tr = cdve(OP_AD2C, u12, u3, sc(76))
    tri = recip(tr, out=newt(name="tri"))
    fs0 = tt(s0, tri, MUL, out=newt(name="fs0"))
    fx0 = tt(x0, tri, MUL, out=newt(name="fx0"))
    fy0 = tt(y0, tri, MUL, out=newt(name="fy0"))
    fs1 = tt(s1, tri, MUL, out=newt(name="fs1"))
    fx1 = tt(x1, tri, MUL, out=newt(name="fx1"))
    fy1 = tt(y1, tri, MUL, out=newt(name="fy1"))

    # M entries: K row-major at consts cols 0..69 (entry e -> cols 7e..7e+6)
    Midx = [(0, 0), (0, 1), (0, 2), (0, 3), (1, 1), (1, 2), (1, 3), (2, 2),
            (2, 3), (3, 3)]
    Mt = {}
    for e in range(10):
        base = 7 * e
        v1_ = cdve(OP_MAD2, fs0, fx0, sc(base + 0), sc(base + 1))
        v2_ = cdve(OP_MAD2, fy0, fs1, sc(base + 2), sc(base + 3))
        v3_ = cdve(OP_MAD2, fx1, fy1, sc(base + 4), sc(base + 5))
        a12 = tt(v1_, v2_, ADD)
        m1 = cdve(OP_MAD1A, tri, a12, sc(base + 6))
        Mt[Midx[e]] = tt(m1, v3_, ADD, out=newt(name=f"M{e}"))

    def Mg(i, j):
        return Mt[(i, j) if i <= j else (j, i)]

    # ---------- Phase B: characteristic polynomial ----------
    t01 = tt(Mg(0, 0), Mg(1, 1), ADD)
    t23 = tt(Mg(2, 2), Mg(3, 3), ADD)
    c3 = tt(t01, t23, ADD, out=newt(name="c3"))
    # c2 = sum over pairs (Mii*Mjj - Mij^2)
    c2 = None
    for (i, j) in [(0, 1), (0, 2), (0, 3), (1, 2), (1, 3), (2, 3)]:
        pr = tt(Mg(i, i), Mg(j, j), MUL)
        term = cdve(OP_SQSUB, Mg(i, j), pr)
        c2 = term if c2 is None else tt(c2, term, ADD)
    c2n = newt(name="c2")
    nc.any.tensor_copy(out=c2n[:], in_=c2[:])
    c2 = c2n

    def det3(rows, cols, G):
        (a, b, c), (d, e, f), (g, h, i_) = [[G(r, cc) for cc in cols] for r in rows]
        # generic det3: a(ei-fh) - b(di-fg) + c(dh-eg)
        ei = tt(e, i_, MUL)
        fh = tt(f, h, MUL)
        m1 = tt(ei, fh, SUB)
        di = tt(d, i_, MUL)
        fg = tt(f, g, MUL)
        m2 = tt(di, fg, SUB)
        dh = tt(d, h, MUL)
        eg = tt(e, g, MUL)
        m3 = tt(dh, eg, SUB)
        r1 = tt(a, m1, MUL)
        r2 = tt(b, m2, MUL)
        r3 = tt(c, m3, MUL)
        s_ = tt(r1, r2, SUB)
        return tt(s_, r3, ADD)

    c1 = None
    for k in range(4):
        rows = [i for i in range(4) if i != k]
        d = det3(rows, rows, Mg)
        c1 = d if c1 is None else tt(c1, d, ADD)
    c1n = newt(name="c1")
    nc.any.tensor_copy(out=c1n[:], in_=c1[:])
    c1 = c1n
    c0 = None
    for j in range(4):
        cols = [c for c in range(4) if c != j]
        d = det3([1, 2, 3], cols, Mg)
        term = tt(Mg(0, j), d, MUL)
        if c0 is None:
            c0 = term
        elif j % 2 == 1:
            c0 = tt(c0, term, SUB)
        else:
            c0 = tt(c0, term, ADD)
    c0n = newt(name="c0")
    nc.any.tensor_copy(out=c0n[:], in_=c0[:])
    c0 = c0n

    # ---------- Phase C: Newton from 0 ----------
    rc1 = recip(c1)
    lam = tt(c0, rc1, MUL)
    for _ in range(NEWTON_ITERS):
        s1_ = cdve(OP_LC, lam, c3)            # (lam - c3) * lam
        s2 = tt(s1_, c2, ADD)
        s3a = tt(s2, lam, MUL)
        s3 = tt(s3a, c1, SUB)
        s4a = tt(s3, lam, MUL)
        pp = tt(s4a, c0, ADD)
        d1 = cdve(OP_MAD2, lam, c3, 4.0, -3.0)
        d2a = tt(d1, lam, MUL)
        d2 = cdve(OP_MAD1A, c2, d2a, 2.0)
        d3a = tt(d2, lam, MUL)
        dp = tt(d3a, c1, SUB)
        rdp = recip(dp)
        st = tt(pp, rdp, MUL)
        lam = tt(lam, st, SUB, out=newt(name=f"lam{next(cnt)}"))

    # ---------- Phase D: eigenvector via 4D crosses ----------
    Bd = {}
    for i in range(4):
        Bd[i] = tt(Mg(i, i), lam, SUB, out=newt(name=f"Bd{i}"))

    def Bg(i, j):
        return Bd[i] if i == j else Mg(i, j)

    # all 2x2 dets of rows (2,3): d2[(a,b)] = B2a*B3b - B2b*B3a for a<b
    d2 = {}
    for (a, b) in [(0, 1), (0, 2), (0, 3), (1, 2), (1, 3), (2, 3)]:
        m1 = tt(Bg(2, a), Bg(3, b), MUL)
        m2 = tt(Bg(2, b), Bg(3, a), MUL)
        d2[(a, b)] = tt(m1, m2, SUB, out=newt(name=f"d2_{a}{b}"))

    def cross_from_row(r0):
        # v_i = (-1)^i * det3(rows [r0,2,3], cols != i), using shared d2
        v = []
        for i in range(4):
            cols = [c for c in range(4) if c != i]
            (ca, cb, cc) = cols
            t1_ = tt(Bg(r0, ca), d2[(cb, cc)], MUL)
            t2_ = tt(Bg(r0, cb), d2[(ca, cc)], MUL)
            t3_ = tt(Bg(r0, cc), d2[(ca, cb)], MUL)
            s_ = tt(t1_, t2_, SUB)
            dd = tt(s_, t3_, ADD)
            v.append(dd)
        # apply cofactor signs: v_i *= (-1)^i — fold into downstream by
        # alternating signs; easier: negate odd entries now.
        v2_ = []
        for i, t in enumerate(v):
            if i % 2 == 1:
                v2_.append(ts(t, -1.0, MUL))
            else:
                v2_.append(t)
        return v2_

    vA = cross_from_row(1)
    vB = cross_from_row(0)
    # row-0-based cross has opposite orientation; sign irrelevant for hom.
    nA1 = cdve(OP_SQSUM2, vA[0], vA[1])
    nA2 = cdve(OP_SQSUM2, vA[2], vA[3])
    nA = tt(nA1, nA2, ADD)
    nB1 = cdve(OP_SQSUM2, vB[0], vB[1])
    nB2 = cdve(OP_SQSUM2, vB[2], vB[3])
    nB = tt(nB1, nB2, ADD)
    msk = newt((P, FD), mybir.dt.uint8, name="msk")
    nc.vector.tensor_tensor(out=msk[:], in0=nA[:], in1=nB[:],
                            op=mybir.AluOpType.is_ge)
    hom = []
    for i in range(4):
        o = newt(name=f"hom{i}")
        nc.vector.select(out=o[:], mask=msk[:], on_true=vA[i][:], on_false=vB[i][:])
        hom.append(o)

    # ---------- Phase E: z, valid, values, stats ----------
    invw = recip(hom[3])
    zs = []
    for v in range(2):
        e0, e1, e2, e3 = (77 + 4 * v, 78 + 4 * v, 79 + 4 * v, 80 + 4 * v)
        sd = ts(hom[0], sc(e0), MUL)
        sd = stt(hom[1], sc(e1), sd, MUL, ADD)
        sd = stt(hom[2], sc(e2), sd, MUL, ADD)
        zr = tt(sd, invw, MUL)
        z = ts(zr, sc(e3), ADD, out=newt(name=f"z{v}"))
        zs.append(z)
    r0 = cdve(OP_RANGE, zs[0], None, TH_MAX)
    r1 = cdve(OP_RANGE, zs[1], None, TH_MAX)
    valid = tt(r0, r1, MUL, out=newt(name="valid"))
    vals = []
    for v in range(2):
        mkz = recip(zs[v])
        vals.append(cdve(OP_SELHALF, mkz, valid, 0.5,
                         out=newt(name=f"vals{v}")))

    # near/far: per-view masked min/max, then partition reduce
    red = newt((P, 8), name="red")
    for v in range(2):
        zmin = cdve(OP_SELC, zs[v], valid, 1e30)
        zmax = cdve(OP_SELC, zs[v], valid, -1e30)
        nc.vector.tensor_reduce(out=red[:, v:v + 1], in_=zmin[:],
                                axis=mybir.AxisListType.X,
                                op=mybir.AluOpType.min, negate=True)
        nc.vector.tensor_reduce(out=red[:, 2 + v:3 + v], in_=zmax[:],
                                axis=mybir.AxisListType.X,
                                op=mybir.AluOpType.max)
    nc.vector.tensor_reduce(out=red[:, 4:5], in_=valid[:],
                            axis=mybir.AxisListType.X, op=mybir.AluOpType.max)
    nc.gpsimd.memset(red[:, 5:8], 0.0)
    # negate=True on the min-reduce writes -min, so max-allreduce works for all
    redall = newt((P, 8), name="redall")
    nc.gpsimd.partition_all_reduce(out_ap=redall[:], in_ap=red[:], channels=P,
                                   reduce_op=bass_isa.ReduceOp.max)
    nc.sync.dma_start(out=stats[:], in_=redall[0:1, :])

    # ---------- Phase F: value routing ----------
    # view 0: expand dense vals into padded-group layout E0
    E0 = newt((P, EW), name="E0")
    nc.gpsimd.local_scatter(
        out_ap=E0[:].bitcast(U16), data_ap=vals[0][:].bitcast(U16),
        idxs_ap=eidx_t[:], channels=P, num_elems=2 * EW, num_idxs=2 * FD)

    # view 1: place into squares (2 halves), PE-transpose, compact into C1
    halves = []
    for hh in range(2):
        Ph = newt((P, SQH * P), name=f"place{hh}")
        nc.gpsimd.local_scatter(
            out_ap=Ph[:].bitcast(U16), data_ap=vals[1][:].bitcast(U16),
            idxs_ap=pidx_t[:, hh * 2 * FD:(hh + 1) * 2 * FD],
            channels=P, num_elems=2 * SQH * P, num_idxs=2 * FD)
        halves.append(Ph)
    ident = newt((P, P), name="ident")
    make_identity(nc, ident[:])
    T1 = newt((P, SQ * P), name="T1")
    for s in range(SQ):
        src = halves[s // SQH]
        off = (s % SQH) * P
        ps = psp.tile([P, P], F32, name=f"ps{s}", tag="ps", bufs=2)
        nc.tensor.transpose(out=ps[:], in_=src[:, off:off + P], identity=ident[:])
        nc.scalar.copy(out=T1[:, s * P:(s + 1) * P], in_=ps[:])
    C1 = newt((P, EW), name="C1")
    nc.gpsimd.local_scatter(
        out_ap=C1[:].bitcast(U16), data_ap=T1[:].bitcast(U16),
        idxs_ap=cidx_t[:], channels=P, num_elems=2 * EW, num_idxs=2 * SQ * P)

    # ---------- Phase G: per-chunk scatter + merge + write out ----------
    EV = [E0, C1]
    SIDX = [sidx0_t, sidx1_t]
    for v in range(2):
        for rb in range(RB):
            for hh in range(CH):
                g = rb * CH + hh
                valc = pool.tile([P, CW], F32, name=f"valc{v}_{g}",
                                 tag="valc", bufs=4)
                nc.gpsimd.local_scatter(
                    out_ap=valc[:].bitcast(U16),
                    data_ap=EV[v][:, g * GW:(g + 1) * GW].bitcast(U16),
                    idxs_ap=SIDX[v][:, g * 2 * GW:(g + 1) * 2 * GW],
                    channels=P, num_elems=2 * CW, num_idxs=2 * GW)
                mslice = mch[v][rb][:, hh * CW:(hh + 1) * CW]
                nc.vector._custom_dve(OP_MERGE, out=mslice, in0=mslice,
                                      in1=valc[:], s0=0.5)
            nc.sync.dma_start(out=out_maps[v, rb * P:(rb + 1) * P, :],
                              in_=mch[v][rb][:])


_NC_CACHE = None


def _get_nc():
    global _NC_CACHE
    if _NC_CACHE is None:
        _NC_CACHE = _build_nc()
    return _NC_CACHE


# ---------------- host-side preparation ----------------

def _host_prep(mkpts0, mkpts1, mconf, mbids, intrinsics, extrinsics,
               fullres_disps):
    f32 = np.float32
    row_scale = np.array([W, H, 1.0])[None, None, :, None]
    intr = intrinsics.astype(np.float64) * row_scale
    extr_inv = np.linalg.inv(extrinsics.astype(np.float64))
    proj = np.einsum('bvij,bvjk->bvik', intr, extr_inv[:, :, :3, :])
    proj32 = proj.astype(f32)
    einv2 = extr_inv[:, :, 2, :].astype(f32)

    disps = fullres_disps.reshape(V, B, H, W)

    in_maps = []
    for b in range(B):
        sel = np.where(mbids == b)[0]
        x0 = mkpts0[sel, 0].astype(f32)
        y0 = mkpts0[sel, 1].astype(f32)
        x1 = mkpts1[sel, 0].astype(f32)
        y1 = mkpts1[sel, 1].astype(f32)
        n = len(sel)
        assert n > 0

        xi0 = np.clip(x0.astype(np.int32), 0, W - 1)
        yi0 = np.clip(y0.astype(np.int32), 0, H - 1)
        xi1 = np.clip(x1.astype(np.int32), 0, W - 1)
        yi1 = np.clip(y1.astype(np.int32), 0, H - 1)

        # last-writer-wins masks per view (in original order)
        def last_writer(yy, xx):
            flat = yy.astype(np.int64) * W + xx
            wmask = np.zeros(n, bool)
            # np.unique keeps first occurrence; reverse for last
            _, idx_rev = np.unique(flat[::-1], return_index=True)
            wmask[n - 1 - idx_rev] = True
            return wmask

        w0 = last_writer(yi0, xi0)
        w1 = last_writer(yi1, xi1)

        part = yi0 % P
        rb0, h0, c0 = yi0 // P, xi0 // CW, xi0 % CW
        rb1, h1, c1 = yi1 // P, xi1 // CW, xi1 % CW
        q = yi1 % P

        # order kps per partition by (group) for compact windows
        order = np.lexsort((rb0 * CH + h0, part))
        slot = np.zeros(n, np.int64)
        pc = np.zeros(P, np.int64)
        for k in order:
            slot[k] = pc[part[k]]
            pc[part[k]] += 1
        assert pc.max() <= FD, f"partition overflow {pc.max()} > {FD}"

        kpt = np.zeros((P, 4 * FD), f32)
        # pad slots with kp 0 (a real keypoint of this scene)
        kpt[:, 0 * FD:1 * FD] = x0[0]
        kpt[:, 1 * FD:2 * FD] = y0[0]
        kpt[:, 2 * FD:3 * FD] = x1[0]
        kpt[:, 3 * FD:4 * FD] = y1[0]
        kpt[part, 0 * FD + slot] = x0
        kpt[part, 1 * FD + slot] = y0
        kpt[part, 2 * FD + slot] = x1
        kpt[part, 3 * FD + slot] = y1

        # expand idx (view 0): dense slot -> padded group slot
        eidx = np.full((P, 2 * FD), -1, np.int16)
        sidx0 = np.full((P, NG * 2 * GW), -1, np.int16)
        gcount0 = np.zeros((P, NG), np.int64)
        for k in np.where(w0)[0]:
            p_, g_ = part[k], rb0[k] * CH + h0[k]
            j = gcount0[p_, g_]
            gcount0[p_, g_] += 1
            assert j < GW, "group overflow v0"
            tgt = g_ * GW + j
            eidx[p_, 2 * slot[k]] = 2 * tgt
            eidx[p_, 2 * slot[k] + 1] = 2 * tgt + 1
            sidx0[p_, g_ * 2 * GW + 2 * j] = 2 * c0[k]
            sidx0[p_, g_ * 2 * GW + 2 * j + 1] = 2 * c0[k] + 1

        # view 1 routing: squares placement, compact, chunk idx
        pidx = np.full((P, 4 * FD), -1, np.int16)
        cidxA = np.full((P, 2 * SQ * P), -1, np.int16)
        sidx1 = np.full((P, NG * 2 * GW), -1, np.int16)
        sqcount = np.zeros((P, P), np.int64)
        gcount1 = np.zeros((P, NG), np.int64)
        for k in np.where(w1)[0]:
            p_, q_ = part[k], q[k]
            s_ = sqcount[p_, q_]
            sqcount[p_, q_] += 1
            assert s_ < SQ, "square overflow"
            hh = 0 if s_ < SQH else 1
            s_in = s_ - hh * SQH
            tgt = s_in * P + q_
            pidx[p_, hh * 2 * FD + 2 * slot[k]] = 2 * tgt
            pidx[p_, hh * 2 * FD + 2 * slot[k] + 1] = 2 * tgt + 1
            # after transpose: at partition q_, f32 col s_*P + p_
            g_ = rb1[k] * CH + h1[k]
            j = gcount1[q_, g_]
            gcount1[q_, g_] += 1
            assert j < GW, "group overflow v1"
            tpos = s_ * P + p_
            ctgt = g_ * GW + j
            cidxA[q_, 2 * tpos] = 2 * ctgt
            cidxA[q_, 2 * tpos + 1] = 2 * ctgt + 1
            sidx1[q_, g_ * 2 * GW + 2 * j] = 2 * c1[k]
            sidx1[q_, g_ * 2 * GW + 2 * j + 1] = 2 * c1[k] + 1

        # consts row
        K = np.zeros((10, 7), f32)
        Midx = [(0, 0), (0, 1), (0, 2), (0, 3), (1, 1), (1, 2), (1, 3),
                (2, 2), (2, 3), (3, 3)]
        for e, (i, j) in enumerate(Midx):
            for v in range(2):
                a = proj32[b, v, 2]
                bb = proj32[b, v, 0]
                cc = proj32[b, v, 1]
                base = 0 if v == 0 else 3
                K[e, base + 0] += a[i] * a[j]
                K[e, base + 1] += -(a[i] * bb[j] + a[j] * bb[i])
                K[e, base + 2] += -(a[i] * cc[j] + a[j] * cc[i])
                K[e, 6] += bb[i] * bb[j] + cc[i] * cc[j]
        Kt = K[[0, 4, 7, 9]].sum(axis=0)
        row = np.zeros(96, f32)
        row[0:70] = K.reshape(-1)
        row[70:77] = Kt
        row[77:81] = einv2[b, 0]
        row[81:85] = einv2[b, 1]
        cstrow = np.tile(row[None, :], (P, 1))

        in_maps.append({
            "kpt": kpt,
            "cst": cstrow,
            "maps": np.ascontiguousarray(disps[:, b]),
            "eidx": eidx,
            "pidx": pidx,
            "cidx": cidxA,
            "sidx0": sidx0,
            "sidx1": sidx1,
        })
    return in_maps


def kernel(mkpts0, mkpts1, mconf, mbids, intrinsics, extrinsics,
           fullres_disps):
    global LAST_RESULTS
    import os
    from concourse.bass_utils import run_bass_kernel_spmd

    nc = _get_nc()
    in_maps = _host_prep(np.asarray(mkpts0), np.asarray(mkpts1),
                         np.asarray(mconf), np.asarray(mbids),
                         np.asarray(intrinsics), np.asarray(extrinsics),
                         np.asarray(fullres_disps))

    res = run_bass_kernel_spmd(nc, in_maps, core_ids=list(range(8)),
                               trace=bool(os.environ.get("KERNEL_TRACE")))
    LAST_RESULTS = res

    fused = np.zeros((V, B, H, W), np.float32)
    near = np.zeros((B, V), np.float32)
    far = np.zeros((B, V), np.float32)
    flag = np.zeros((B, V), np.float32)
    for b in range(B):
        r = res.results[b]
        fused[:, b] = r["out_maps"]
        st = r["stats"][0]
        has = st[4] > 0
        for v in range(2):
            near[b, v] = -st[v] if has else 0.0
            far[b, v] = st[2 + v] if has else 500.0
            flag[b, v] = 1.0 if has else 0.0
    return (fused.reshape(V * B, 1, H, W), near, far, flag)


# revision 6
# speedup vs baseline: 1.1000x; 1.1000x over previous
"""Trainium2 Bass kernel for nn_DepthPredictorMultiView.

Self-contained: takes FULL inputs (as produced by the reference's
setup_inputs), shards per scene across 8 NeuronCores, runs a Bass/Tile
kernel per core, gathers the full outputs.

Per-core device pipeline (scene-parallel):
  - DLT triangulation per keypoint: M = A^T A (4x4 symmetric, built from a
    host-precomputed 10x7 quadratic-form matrix), smallest eigenvalue via
    Newton on the characteristic quartic, eigenvector via 4D cross products.
  - z per view from the inverse-extrinsics row, validity, near/far
    reductions.
  - Disparity fusion: maps stream through SBUF; fused values are placed
    into dense per-chunk VAL tiles with GPSIMD local_scatter (f32 values
    scattered as interleaved u16 pairs; cross-partition routing for view 1
    via PE transposes), then merged elementwise and written out.
"""

import itertools
import numpy as np

import concourse.bass as bass
import concourse.bacc as bacc
import concourse.mybir as mybir
import concourse.dve_ops as dve_ops
from concourse.dve_ops import DveOp, has_src1
from concourse.dve_spec import Spec, Src0, Src1, C0, C1, Zero, One, select, sq, lower
from concourse.dve_uop import DveOpSpec
from concourse.tile import TileContext
from concourse.masks import make_identity
from concourse import bass_isa

# ---------------- problem constants (hardcoded) ----------------
B, V, H, W = 8, 2, 1024, 1024
N = 131072
TH_MAX = 500.0
P = 128          # partitions
RB = 8           # row blocks per map (1024/128)
CH = 2           # column halves per row block
CW = 512         # chunk width in f32
FD = 176         # keypoint slots per partition
GW = 24          # padded per-(partition, chunk) group width
NG = RB * CH     # 16 groups per view
EW = NG * GW     # 384: expanded/compacted value layout width
SQ = 10          # transpose squares for view-1 routing
SQH = SQ // 2
NEWTON_ITERS = 7
TINY = 1e-30

F32 = mybir.dt.float32
I16 = mybir.dt.int16
U16 = mybir.dt.uint16

LAST_RESULTS = None     # BassKernelResults of the most recent run (for test.py)

# ---------------- custom DVE ops ----------------
_REGISTERED = {}


def _reg(name, body, reference):
    if name in _REGISTERED:
        return _REGISTERED[name]
    spec = Spec(body=body, reference=reference)
    op = DveOp(name, spec, subdim=False, uops_sha={})
    dve_ops.OPS.append(op)
    dve_ops.CUSTOM_DVE_SPECS[name] = spec
    row = max(dve_ops._SUB_OPCODE_FOR_NAME.values()) + 1
    assert row < 0x20
    dve_ops._SUB_OPCODE_FOR_NAME[name] = row
    for ver in ("v3", "v4"):
        s = DveOpSpec(
            name=name,
            opcode=row,
            uops=lower(spec, ver=ver),
            rd1_en=has_src1(spec),
        )
        op.uops_sha[ver] = s.sha(ver)
    _REGISTERED[name] = op
    return op


OP_SQSUM2 = _reg(
    "ANT_SQSUM2", sq(Src0) + sq(Src1),
    lambda in0, in1, s0, s1, imm2: (in0 * in0 + in1 * in1).astype(np.float32),
)
OP_MAD2 = _reg(
    "ANT_MAD2", Src0 * C0 + Src1 * C1,
    lambda in0, in1, s0, s1, imm2: (in0 * s0 + in1 * s1).astype(np.float32),
)
OP_MAD1A = _reg(
    "ANT_MAD1A", Src0 * C0 + Src1,
    lambda in0, in1, s0, s1, imm2: (in0 * s0 + in1).astype(np.float32),
)
OP_AD2C = _reg(
    "ANT_AD2C", Src0 + Src1 + C0,
    lambda in0, in1, s0, s1, imm2: (in0 + in1 + s0).astype(np.float32),
)
OP_SQSUB = _reg(
    "ANT_SQSUB", Src1 - sq(Src0),
    lambda in0, in1, s0, s1, imm2: (in1 - in0 * in0).astype(np.float32),
)
OP_LC = _reg(
    "ANT_LC", (Src0 - Src1) * Src0,
    lambda in0, in1, s0, s1, imm2: ((in0 - in1) * in0).astype(np.float32),
)
OP_RANGE = _reg(
    "ANT_RANGE", (Src0 > Zero) & (Src0 < C0),
    lambda in0, in1, s0, s1, imm2: ((in0 > 0) & (in0 < s0)).astype(np.float32),
)
OP_SELHALF = _reg(
    "ANT_SELHALF", select(Src1 > Zero, Src0 * C0, Zero),
    lambda in0, in1, s0, s1, imm2: np.where(in1 > 0, in0 * s0, 0.0).astype(np.float32),
)
OP_SELC = _reg(
    "ANT_SELC", select(Src1 > Zero, Src0, C0),
    lambda in0, in1, s0, s1, imm2: np.where(in1 > 0, in0, s0).astype(np.float32),
)
OP_MERGE = _reg(
    "ANT_MERGE", Src0 - Src0 * (Src1 > Zero) * C0 + Src1,
    lambda in0, in1, s0, s1, imm2: (in0 - in0 * (in1 > 0) * s0 + in1).astype(np.float32),
)
OP_MSUB = _reg(
    "ANT_MSUB", Src0 * Src1,
    lambda in0, in1, s0, s1, imm2: (in0 * in1).astype(np.float32),
)


# ---------------- builder ----------------

def _build_nc():
    nc = bacc.Bacc()

    kpt = nc.declare_dram_parameter("kpt", [P, 4 * FD], F32, isOutput=False)
    cst = nc.declare_dram_parameter("cst", [P, 96], F32, isOutput=False)
    maps = nc.declare_dram_parameter("maps", [V, H, W], F32, isOutput=False)
    eidx = nc.declare_dram_parameter("eidx", [P, 2 * FD], I16, isOutput=False)
    pidx = nc.declare_dram_parameter("pidx", [P, 4 * FD], I16, isOutput=False)
    cidx = nc.declare_dram_parameter("cidx", [P, 2 * SQ * P], I16, isOutput=False)
    sidx0 = nc.declare_dram_parameter("sidx0", [P, NG * 2 * GW], I16, isOutput=False)
    sidx1 = nc.declare_dram_parameter("sidx1", [P, NG * 2 * GW], I16, isOutput=False)

    out_maps = nc.declare_dram_parameter("out_maps", [V, H, W], F32, isOutput=True)
    stats = nc.declare_dram_parameter("stats", [1, 8], F32, isOutput=True)

    with TileContext(nc) as tc:
        with (
            tc.tile_pool(name="main", bufs=1) as pool,
            tc.tile_pool(name="psum", bufs=2, space="PSUM") as psp,
        ):
            _emit(nc, tc, pool, psp, kpt, cst, maps, eidx, pidx, cidx, sidx0,
                  sidx1, out_maps, stats)
    nc.finalize()
    return nc


def _emit(nc, tc, pool, psp, kpt, cst, maps, eidx, pidx, cidx, sidx0, sidx1,
          out_maps, stats):
    cnt = itertools.count()

    def newt(shape=(P, FD), dtype=F32, name=None):
        if name is None:
            return pool.tile(list(shape), dtype, name=f"w{next(cnt)}",
                             tag="work", bufs=64)
        return pool.tile(list(shape), dtype, name=name)

    _ARITH = {mybir.AluOpType.mult, mybir.AluOpType.add,
              mybir.AluOpType.subtract, mybir.AluOpType.max,
              mybir.AluOpType.min}

    def tt(a, b, op, out=None):
        o = out if out is not None else newt()
        eng = nc.any if op in _ARITH else nc.vector
        eng.tensor_tensor(out=o[:], in0=a[:], in1=b[:], op=op)
        return o

    def ts(a, s1, op0, s2=None, op1=None, out=None):
        o = out if out is not None else newt()
        eng = nc.any if op0 in _ARITH and (op1 is None or op1 in _ARITH) \
            else nc.vector
        eng.tensor_scalar(
            out=o[:], in0=a[:], scalar1=s1, scalar2=s2,
            op0=op0, op1=op1 if op1 is not None else mybir.AluOpType.bypass)
        return o

    def stt(a, s, b, op0, op1, out=None):
        o = out if out is not None else newt()
        nc.vector.scalar_tensor_tensor(
            out=o[:], in0=a[:], scalar=s, in1=b[:], op0=op0, op1=op1)
        return o

    def cdve(op, in0, in1=None, s0=0.0, s1=0.0, out=None, shape=(P, FD)):
        o = out if out is not None else newt(shape)
        nc.vector._custom_dve(
            op, out=o[:], in0=in0[:],
            in1=in1[:] if in1 is not None else None, s0=s0, s1=s1)
        return o

    def recip(a, out=None, fast=False):
        o = out if out is not None else newt()
        if fast:
            nc.vector.reciprocal_approx_fast(out=o[:], in_=a[:])
        else:
            nc.vector.reciprocal(out=o[:], in_=a[:])
        return o

    MUL = mybir.AluOpType.mult
    ADD = mybir.AluOpType.add
    SUB = mybir.AluOpType.subtract

    # ---------- DMA in ----------
    kp = newt((P, 4 * FD), name="kp")
    nc.sync.dma_start(out=kp[:], in_=kpt[:])
    cstt = newt((P, 96), name="cstt")
    nc.sync.dma_start(out=cstt[:], in_=cst[:])
    sc = lambda j: cstt[:, j:j + 1]  # noqa: E731

    eidx_t = newt((P, 2 * FD), I16, name="eidx_t")
    nc.sync.dma_start(out=eidx_t[:], in_=eidx[:])
    pidx_t = newt((P, 4 * FD), I16, name="pidx_t")
    nc.sync.dma_start(out=pidx_t[:], in_=pidx[:])
    cidx_t = newt((P, 2 * SQ * P), I16, name="cidx_t")
    nc.sync.dma_start(out=cidx_t[:], in_=cidx[:])
    sidx0_t = newt((P, NG * 2 * GW), I16, name="sidx0_t")
    nc.sync.dma_start(out=sidx0_t[:], in_=sidx0[:])
    sidx1_t = newt((P, NG * 2 * GW), I16, name="sidx1_t")
    nc.sync.dma_start(out=sidx1_t[:], in_=sidx1[:])

    mch = []  # map chunks (v, rb) -> (128, 1024) tile
    for v in range(V):
        row = []
        for rb in range(RB):
            t = newt((P, W), name=f"mch{v}_{rb}")
            nc.sync.dma_start(out=t[:], in_=maps[v, rb * P:(rb + 1) * P, :])
            row.append(t)
        mch.append(row)

    x0 = kp[:, 0 * FD:1 * FD]
    y0 = kp[:, 1 * FD:2 * FD]
    x1 = kp[:, 2 * FD:3 * FD]
    y1 = kp[:, 3 * FD:4 * FD]

    class A:  # AP wrapper so helpers can take raw slices uniformly
        def __init__(self, ap):
            self.ap = ap

        def __getitem__(self, k):
            return self.ap

    x0, y0, x1, y1 = A(x0), A(y0), A(x1), A(y1)

    # ---------- Phase A: features, trace, M ----------
    s0 = cdve(OP_SQSUM2, x0, y0)
    s1 = cdve(OP_SQSUM2, x1, y1)
    # trace = Kt . feats + Kt6  (Kt at consts cols 70..76)
    u1 = cdve(OP_MAD2, s0, x0, sc(70), sc(71))
    u2 = cdve(OP_MAD2, y0, s1, sc(72), sc(73))
    u3 = cdve(OP_MAD2, x1, y1, sc(74), sc(75))
    u12 = tt(u1, u2, ADD)
    tr = cdve(OP_AD2C, u12, u3, sc(76))
    tri = recip(tr, out=newt(name="tri"), fast=True)
    fs0 = tt(s0, tri, MUL, out=newt(name="fs0"))
    fx0 = tt(x0, tri, MUL, out=newt(name="fx0"))
    fy0 = tt(y0, tri, MUL, out=newt(name="fy0"))
    fs1 = tt(s1, tri, MUL, out=newt(name="fs1"))
    fx1 = tt(x1, tri, MUL, out=newt(name="fx1"))
    fy1 = tt(y1, tri, MUL, out=newt(name="fy1"))

    # M entries: K row-major at consts cols 0..69 (entry e -> cols 7e..7e+6)
    Midx = [(0, 0), (0, 1), (0, 2), (0, 3), (1, 1), (1, 2), (1, 3), (2, 2),
            (2, 3), (3, 3)]
    Mt = {}
    for e in range(10):
        base = 7 * e
        v1_ = cdve(OP_MAD2, fs0, fx0, sc(base + 0), sc(base + 1))
        v2_ = cdve(OP_MAD2, fy0, fs1, sc(base + 2), sc(base + 3))
        v3_ = cdve(OP_MAD2, fx1, fy1, sc(base + 4), sc(base + 5))
        a12 = tt(v1_, v2_, ADD)
        m1 = cdve(OP_MAD1A, tri, a12, sc(base + 6))
        Mt[Midx[e]] = tt(m1, v3_, ADD, out=newt(name=f"M{e}"))

    def Mg(i, j):
        return Mt[(i, j) if i <= j else (j, i)]

    # ---------- Phase B: characteristic polynomial ----------
    t01 = tt(Mg(0, 0), Mg(1, 1), ADD)
    t23 = tt(Mg(2, 2), Mg(3, 3), ADD)
    c3 = tt(t01, t23, ADD, out=newt(name="c3"))
    # c2 = sum over pairs (Mii*Mjj - Mij^2)
    c2 = None
    for (i, j) in [(0, 1), (0, 2), (0, 3), (1, 2), (1, 3), (2, 3)]:
        pr = tt(Mg(i, i), Mg(j, j), MUL)
        term = cdve(OP_SQSUB, Mg(i, j), pr)
        c2 = term if c2 is None else tt(c2, term, ADD)
    c2n = newt(name="c2")
    nc.any.tensor_copy(out=c2n[:], in_=c2[:])
    c2 = c2n

    def det3(rows, cols, G):
        (a, b, c), (d, e, f), (g, h, i_) = [[G(r, cc) for cc in cols] for r in rows]
        # generic det3: a(ei-fh) - b(di-fg) + c(dh-eg)
        ei = tt(e, i_, MUL)
        fh = tt(f, h, MUL)
        m1 = tt(ei, fh, SUB)
        di = tt(d, i_, MUL)
        fg = tt(f, g, MUL)
        m2 = tt(di, fg, SUB)
        dh = tt(d, h, MUL)
        eg = tt(e, g, MUL)
        m3 = tt(dh, eg, SUB)
        r1 = tt(a, m1, MUL)
        r2 = tt(b, m2, MUL)
        r3 = tt(c, m3, MUL)
        s_ = tt(r1, r2, SUB)
        return tt(s_, r3, ADD)

    c1 = None
    for k in range(4):
        rows = [i for i in range(4) if i != k]
        d = det3(rows, rows, Mg)
        c1 = d if c1 is None else tt(c1, d, ADD)
    c1n = newt(name="c1")
    nc.any.tensor_copy(out=c1n[:], in_=c1[:])
    c1 = c1n
    c0 = None
    for j in range(4):
        cols = [c for c in range(4) if c != j]
        d = det3([1, 2, 3], cols, Mg)
        term = tt(Mg(0, j), d, MUL)
        if c0 is None:
            c0 = term
        elif j % 2 == 1:
            c0 = tt(c0, term, SUB)
        else:
            c0 = tt(c0, term, ADD)
    c0n = newt(name="c0")
    nc.any.tensor_copy(out=c0n[:], in_=c0[:])
    c0 = c0n

    # ---------- Phase C: Newton from 0 ----------
    rc1 = recip(c1, fast=True)
    lam = tt(c0, rc1, MUL)
    for _ in range(NEWTON_ITERS):
        s1_ = cdve(OP_LC, lam, c3)            # (lam - c3) * lam
        s2 = tt(s1_, c2, ADD)
        s3a = tt(s2, lam, MUL)
        s3 = tt(s3a, c1, SUB)
        s4a = tt(s3, lam, MUL)
        pp = tt(s4a, c0, ADD)
        d1 = cdve(OP_MAD2, lam, c3, 4.0, -3.0)
        d2a = tt(d1, lam, MUL)
        d2 = cdve(OP_MAD1A, c2, d2a, 2.0)
        d3a = tt(d2, lam, MUL)
        dp = tt(d3a, c1, SUB)
        rdp = recip(dp, fast=True)
        st = tt(pp, rdp, MUL)
        lam = tt(lam, st, SUB, out=newt(name=f"lam{next(cnt)}"))

    # ---------- Phase D: eigenvector via 4D crosses ----------
    Bd = {}
    for i in range(4):
        Bd[i] = tt(Mg(i, i), lam, SUB, out=newt(name=f"Bd{i}"))

    def Bg(i, j):
        return Bd[i] if i == j else Mg(i, j)

    # all 2x2 dets of rows (2,3): d2[(a,b)] = B2a*B3b - B2b*B3a for a<b
    d2 = {}
    for (a, b) in [(0, 1), (0, 2), (0, 3), (1, 2), (1, 3), (2, 3)]:
        m1 = tt(Bg(2, a), Bg(3, b), MUL)
        m2 = tt(Bg(2, b), Bg(3, a), MUL)
        d2[(a, b)] = tt(m1, m2, SUB, out=newt(name=f"d2_{a}{b}"))

    def cross_from_row(r0):
        # v_i = (-1)^i * det3(rows [r0,2,3], cols != i), using shared d2
        v = []
        for i in range(4):
            cols = [c for c in range(4) if c != i]
            (ca, cb, cc) = cols
            t1_ = tt(Bg(r0, ca), d2[(cb, cc)], MUL)
            t2_ = tt(Bg(r0, cb), d2[(ca, cc)], MUL)
            t3_ = tt(Bg(r0, cc), d2[(ca, cb)], MUL)
            s_ = tt(t1_, t2_, SUB)
            dd = tt(s_, t3_, ADD)
            v.append(dd)
        # apply cofactor signs: v_i *= (-1)^i — fold into downstream by
        # alternating signs; easier: negate odd entries now.
        v2_ = []
        for i, t in enumerate(v):
            if i % 2 == 1:
                v2_.append(ts(t, -1.0, MUL))
            else:
                v2_.append(t)
        return v2_

    vA = cross_from_row(1)
    vB = cross_from_row(0)
    # row-0-based cross has opposite orientation; sign irrelevant for hom.
    nA1 = cdve(OP_SQSUM2, vA[0], vA[1])
    nA2 = cdve(OP_SQSUM2, vA[2], vA[3])
    nA = tt(nA1, nA2, ADD)
    nB1 = cdve(OP_SQSUM2, vB[0], vB[1])
    nB2 = cdve(OP_SQSUM2, vB[2], vB[3])
    nB = tt(nB1, nB2, ADD)
    msk = newt((P, FD), mybir.dt.uint8, name="msk")
    nc.vector.tensor_tensor(out=msk[:], in0=nA[:], in1=nB[:],
                            op=mybir.AluOpType.is_ge)
    hom = []
    for i in range(4):
        o = newt(name=f"hom{i}")
        nc.vector.select(out=o[:], mask=msk[:], on_true=vA[i][:], on_false=vB[i][:])
        hom.append(o)

    # ---------- Phase E: z, valid, values, stats ----------
    invw = recip(hom[3])
    zs = []
    for v in range(2):
        e0, e1, e2, e3 = (77 + 4 * v, 78 + 4 * v, 79 + 4 * v, 80 + 4 * v)
        sd = ts(hom[0], sc(e0), MUL)
        sd = stt(hom[1], sc(e1), sd, MUL, ADD)
        sd = stt(hom[2], sc(e2), sd, MUL, ADD)
        zr = tt(sd, invw, MUL)
        z = ts(zr, sc(e3), ADD, out=newt(name=f"z{v}"))
        zs.append(z)
    r0 = cdve(OP_RANGE, zs[0], None, TH_MAX)
    r1 = cdve(OP_RANGE, zs[1], None, TH_MAX)
    valid = tt(r0, r1, MUL, out=newt(name="valid"))
    vals = []
    for v in range(2):
        mkz = recip(zs[v])
        vals.append(cdve(OP_SELHALF, mkz, valid, 0.5,
                         out=newt(name=f"vals{v}")))

    # near/far: per-view masked min/max, then partition reduce
    red = newt((P, 8), name="red")
    for v in range(2):
        zmin = cdve(OP_SELC, zs[v], valid, 1e30)
        zmax = cdve(OP_SELC, zs[v], valid, -1e30)
        nc.vector.tensor_reduce(out=red[:, v:v + 1], in_=zmin[:],
                                axis=mybir.AxisListType.X,
                                op=mybir.AluOpType.min, negate=True)
        nc.vector.tensor_reduce(out=red[:, 2 + v:3 + v], in_=zmax[:],
                                axis=mybir.AxisListType.X,
                                op=mybir.AluOpType.max)
    nc.vector.tensor_reduce(out=red[:, 4:5], in_=valid[:],
                            axis=mybir.AxisListType.X, op=mybir.AluOpType.max)
    nc.gpsimd.memset(red[:, 5:8], 0.0)
    # negate=True on the min-reduce writes -min, so max-allreduce works for all
    redall = newt((P, 8), name="redall")
    nc.gpsimd.partition_all_reduce(out_ap=redall[:], in_ap=red[:], channels=P,
                                   reduce_op=bass_isa.ReduceOp.max)
    nc.sync.dma_start(out=stats[:], in_=redall[0:1, :])

    # ---------- Phase F: value routing ----------
    # view 0: expand dense vals into padded-group layout E0
    E0 = newt((P, EW), name="E0")
    nc.gpsimd.local_scatter(
        out_ap=E0[:].bitcast(U16), data_ap=vals[0][:].bitcast(U16),
        idxs_ap=eidx_t[:], channels=P, num_elems=2 * EW, num_idxs=2 * FD)

    # view 1: place into squares (2 halves), PE-transpose, compact into C1
    halves = []
    for hh in range(2):
        Ph = newt((P, SQH * P), name=f"place{hh}")
        nc.gpsimd.local_scatter(
            out_ap=Ph[:].bitcast(U16), data_ap=vals[1][:].bitcast(U16),
            idxs_ap=pidx_t[:, hh * 2 * FD:(hh + 1) * 2 * FD],
            channels=P, num_elems=2 * SQH * P, num_idxs=2 * FD)
        halves.append(Ph)
    ident = newt((P, P), name="ident")
    make_identity(nc, ident[:])
    T1 = newt((P, SQ * P), name="T1")
    for s in range(SQ):
        src = halves[s // SQH]
        off = (s % SQH) * P
        ps = psp.tile([P, P], F32, name=f"ps{s}", tag="ps", bufs=2)
        nc.tensor.transpose(out=ps[:], in_=src[:, off:off + P], identity=ident[:])
        nc.scalar.copy(out=T1[:, s * P:(s + 1) * P], in_=ps[:])
    C1 = newt((P, EW), name="C1")
    nc.gpsimd.local_scatter(
        out_ap=C1[:].bitcast(U16), data_ap=T1[:].bitcast(U16),
        idxs_ap=cidx_t[:], channels=P, num_elems=2 * EW, num_idxs=2 * SQ * P)

    # ---------- Phase G: per-chunk scatter + merge + write out ----------
    EV = [E0, C1]
    SIDX = [sidx0_t, sidx1_t]
    for v in range(2):
        for rb in range(RB):
            for hh in range(CH):
                g = rb * CH + hh
                valc = pool.tile([P, CW], F32, name=f"valc{v}_{g}",
                                 tag="valc", bufs=4)
                nc.gpsimd.local_scatter(
                    out_ap=valc[:].bitcast(U16),
                    data_ap=EV[v][:, g * GW:(g + 1) * GW].bitcast(U16),
                    idxs_ap=SIDX[v][:, g * 2 * GW:(g + 1) * 2 * GW],
                    channels=P, num_elems=2 * CW, num_idxs=2 * GW)
                mslice = mch[v][rb][:, hh * CW:(hh + 1) * CW]
                nc.vector._custom_dve(OP_MERGE, out=mslice, in0=mslice,
                                      in1=valc[:], s0=0.5)
            nc.sync.dma_start(out=out_maps[v, rb * P:(rb + 1) * P, :],
                              in_=mch[v][rb][:])


_NC_CACHE = None


def _get_nc():
    global _NC_CACHE
    if _NC_CACHE is None:
        _NC_CACHE = _build_nc()
    return _NC_CACHE


# ---------------- host-side preparation ----------------

def _host_prep(mkpts0, mkpts1, mconf, mbids, intrinsics, extrinsics,
               fullres_disps):
    f32 = np.float32
    row_scale = np.array([W, H, 1.0])[None, None, :, None]
    intr = intrinsics.astype(np.float64) * row_scale
    extr_inv = np.linalg.inv(extrinsics.astype(np.float64))
    proj = np.einsum('bvij,bvjk->bvik', intr, extr_inv[:, :, :3, :])
    proj32 = proj.astype(f32)
    einv2 = extr_inv[:, :, 2, :].astype(f32)

    disps = fullres_disps.reshape(V, B, H, W)

    in_maps = []
    for b in range(B):
        sel = np.where(mbids == b)[0]
        x0 = mkpts0[sel, 0].astype(f32)
        y0 = mkpts0[sel, 1].astype(f32)
        x1 = mkpts1[sel, 0].astype(f32)
        y1 = mkpts1[sel, 1].astype(f32)
        n = len(sel)
        assert n > 0

        xi0 = np.clip(x0.astype(np.int32), 0, W - 1)
        yi0 = np.clip(y0.astype(np.int32), 0, H - 1)
        xi1 = np.clip(x1.astype(np.int32), 0, W - 1)
        yi1 = np.clip(y1.astype(np.int32), 0, H - 1)

        # last-writer-wins masks per view (in original order)
        def last_writer(yy, xx):
            flat = yy.astype(np.int64) * W + xx
            wmask = np.zeros(n, bool)
            # np.unique keeps first occurrence; reverse for last
            _, idx_rev = np.unique(flat[::-1], return_index=True)
            wmask[n - 1 - idx_rev] = True
            return wmask

        w0 = last_writer(yi0, xi0)
        w1 = last_writer(yi1, xi1)

        part = yi0 % P
        rb0, h0, c0 = yi0 // P, xi0 // CW, xi0 % CW
        rb1, h1, c1 = yi1 // P, xi1 // CW, xi1 % CW
        q = yi1 % P

        # order kps per partition by (group) for compact windows
        order = np.lexsort((rb0 * CH + h0, part))
        slot = np.zeros(n, np.int64)
        pc = np.zeros(P, np.int64)
        for k in order:
            slot[k] = pc[part[k]]
            pc[part[k]] += 1
        assert pc.max() <= FD, f"partition overflow {pc.max()} > {FD}"

        kpt = np.zeros((P, 4 * FD), f32)
        # pad slots with kp 0 (a real keypoint of this scene)
        kpt[:, 0 * FD:1 * FD] = x0[0]
        kpt[:, 1 * FD:2 * FD] = y0[0]
        kpt[:, 2 * FD:3 * FD] = x1[0]
        kpt[:, 3 * FD:4 * FD] = y1[0]
        kpt[part, 0 * FD + slot] = x0
        kpt[part, 1 * FD + slot] = y0
        kpt[part, 2 * FD + slot] = x1
        kpt[part, 3 * FD + slot] = y1

        # expand idx (view 0): dense slot -> padded group slot
        eidx = np.full((P, 2 * FD), -1, np.int16)
        sidx0 = np.full((P, NG * 2 * GW), -1, np.int16)
        gcount0 = np.zeros((P, NG), np.int64)
        for k in np.where(w0)[0]:
            p_, g_ = part[k], rb0[k] * CH + h0[k]
            j = gcount0[p_, g_]
            gcount0[p_, g_] += 1
            assert j < GW, "group overflow v0"
            tgt = g_ * GW + j
            eidx[p_, 2 * slot[k]] = 2 * tgt
            eidx[p_, 2 * slot[k] + 1] = 2 * tgt + 1
            sidx0[p_, g_ * 2 * GW + 2 * j] = 2 * c0[k]
            sidx0[p_, g_ * 2 * GW + 2 * j + 1] = 2 * c0[k] + 1

        # view 1 routing: squares placement, compact, chunk idx
        pidx = np.full((P, 4 * FD), -1, np.int16)
        cidxA = np.full((P, 2 * SQ * P), -1, np.int16)
        sidx1 = np.full((P, NG * 2 * GW), -1, np.int16)
        sqcount = np.zeros((P, P), np.int64)
        gcount1 = np.zeros((P, NG), np.int64)
        for k in np.where(w1)[0]:
            p_, q_ = part[k], q[k]
            s_ = sqcount[p_, q_]
            sqcount[p_, q_] += 1
            assert s_ < SQ, "square overflow"
            hh = 0 if s_ < SQH else 1
            s_in = s_ - hh * SQH
            tgt = s_in * P + q_
            pidx[p_, hh * 2 * FD + 2 * slot[k]] = 2 * tgt
            pidx[p_, hh * 2 * FD + 2 * slot[k] + 1] = 2 * tgt + 1
            # after transpose: at partition q_, f32 col s_*P + p_
            g_ = rb1[k] * CH + h1[k]
            j = gcount1[q_, g_]
            gcount1[q_, g_] += 1
            assert j < GW, "group overflow v1"
            tpos = s_ * P + p_
            ctgt = g_ * GW + j
            cidxA[q_, 2 * tpos] = 2 * ctgt
            cidxA[q_, 2 * tpos + 1] = 2 * ctgt + 1
            sidx1[q_, g_ * 2 * GW + 2 * j] = 2 * c1[k]
            sidx1[q_, g_ * 2 * GW + 2 * j + 1] = 2 * c1[k] + 1

        # consts row
        K = np.zeros((10, 7), f32)
        Midx = [(0, 0), (0, 1), (0, 2), (0, 3), (1, 1), (1, 2), (1, 3),
                (2, 2), (2, 3), (3, 3)]
        for e, (i, j) in enumerate(Midx):
            for v in range(2):
                a = proj32[b, v, 2]
                bb = proj32[b, v, 0]
                cc = proj32[b, v, 1]
                base = 0 if v == 0 else 3
                K[e, base + 0] += a[i] * a[j]
                K[e, base + 1] += -(a[i] * bb[j] + a[j] * bb[i])
                K[e, base + 2] += -(a[i] * cc[j] + a[j] * cc[i])
                K[e, 6] += bb[i] * bb[j] + cc[i] * cc[j]
        Kt = K[[0, 4, 7, 9]].sum(axis=0)
        row = np.zeros(96, f32)
        row[0:70] = K.reshape(-1)
        row[70:77] = Kt
        row[77:81] = einv2[b, 0]
        row[81:85] = einv2[b, 1]
        cstrow = np.tile(row[None, :], (P, 1))

        in_maps.append({
            "kpt": kpt,
            "cst": cstrow,
            "maps": np.ascontiguousarray(disps[:, b]),
            "eidx": eidx,
            "pidx": pidx,
            "cidx": cidxA,
            "sidx0": sidx0,
            "sidx1": sidx1,
        })
    return in_maps


def kernel(mkpts0, mkpts1, mconf, mbids, intrinsics, extrinsics,
           fullres_disps):
    global LAST_RESULTS
    import os
    from concourse.bass_utils import run_bass_kernel_spmd

    nc = _get_nc()
    in_maps = _host_prep(np.asarray(mkpts0), np.asarray(mkpts1),
                         np.asarray(mconf), np.asarray(mbids),
                         np.asarray(intrinsics), np.asarray(extrinsics),
                         np.asarray(fullres_disps))

    res = run_bass_kernel_spmd(nc, in_maps, core_ids=list(range(8)),
                               trace=bool(os.environ.get("KERNEL_TRACE")))
    LAST_RESULTS = res

    fused = np.zeros((V, B, H, W), np.float32)
    near = np.zeros((B, V), np.float32)
    far = np.zeros((B, V), np.float32)
    flag = np.zeros((B, V), np.float32)
    for b in range(B):
        r = res.results[b]
        fused[:, b] = r["out_maps"]
        st = r["stats"][0]
        has = st[4] > 0
        for v in range(2):
            near[b, v] = -st[v] if has else 0.0
            far[b, v] = st[2 + v] if has else 500.0
            flag[b, v] = 1.0 if has else 0.0
    return (fused.reshape(V * B, 1, H, W), near, far, flag)


# revision 7
# speedup vs baseline: 1.1172x; 1.0157x over previous
"""Trainium2 Bass kernel for nn_DepthPredictorMultiView.

Self-contained: takes FULL inputs (as produced by the reference's
setup_inputs), shards per scene across 8 NeuronCores, runs a Bass/Tile
kernel per core, gathers the full outputs.

Per-core device pipeline (scene-parallel):
  - DLT triangulation per keypoint: M = A^T A (4x4 symmetric, built from a
    host-precomputed 10x7 quadratic-form matrix), smallest eigenvalue via
    Newton on the characteristic quartic, eigenvector via 4D cross products.
  - z per view from the inverse-extrinsics row, validity, near/far
    reductions.
  - Disparity fusion: maps stream through SBUF; fused values are placed
    into dense per-chunk VAL tiles with GPSIMD local_scatter (f32 values
    scattered as interleaved u16 pairs; cross-partition routing for view 1
    via PE transposes), then merged elementwise and written out.
"""

import itertools
import numpy as np

import concourse.bass as bass
import concourse.bacc as bacc
import concourse.mybir as mybir
import concourse.dve_ops as dve_ops
from concourse.dve_ops import DveOp, has_src1
from concourse.dve_spec import Spec, Src0, Src1, C0, C1, Zero, One, select, sq, lower
from concourse.dve_uop import DveOpSpec
from concourse.tile import TileContext
from concourse.masks import make_identity
from concourse import bass_isa

# ---------------- problem constants (hardcoded) ----------------
B, V, H, W = 8, 2, 1024, 1024
N = 131072
TH_MAX = 500.0
P = 128          # partitions
RB = 8           # row blocks per map (1024/128)
CH = 2           # column halves per row block
CW = 512         # chunk width in f32
FD = 176         # keypoint slots per partition
GW = 24          # padded per-(partition, chunk) group width
NG = RB * CH     # 16 groups per view
EW = NG * GW     # 384: expanded/compacted value layout width
SQ = 10          # transpose squares for view-1 routing
SQH = SQ // 2
NEWTON_ITERS = 7
TINY = 1e-30

F32 = mybir.dt.float32
I16 = mybir.dt.int16
U16 = mybir.dt.uint16

LAST_RESULTS = None     # BassKernelResults of the most recent run (for test.py)

# ---------------- custom DVE ops ----------------
_REGISTERED = {}


def _reg(name, body, reference):
    if name in _REGISTERED:
        return _REGISTERED[name]
    spec = Spec(body=body, reference=reference)
    op = DveOp(name, spec, subdim=False, uops_sha={})
    dve_ops.OPS.append(op)
    dve_ops.CUSTOM_DVE_SPECS[name] = spec
    row = max(dve_ops._SUB_OPCODE_FOR_NAME.values()) + 1
    assert row < 0x20
    dve_ops._SUB_OPCODE_FOR_NAME[name] = row
    for ver in ("v3", "v4"):
        s = DveOpSpec(
            name=name,
            opcode=row,
            uops=lower(spec, ver=ver),
            rd1_en=has_src1(spec),
        )
        op.uops_sha[ver] = s.sha(ver)
    _REGISTERED[name] = op
    return op


OP_SQSUM2 = _reg(
    "ANT_SQSUM2", sq(Src0) + sq(Src1),
    lambda in0, in1, s0, s1, imm2: (in0 * in0 + in1 * in1).astype(np.float32),
)
OP_MAD2 = _reg(
    "ANT_MAD2", Src0 * C0 + Src1 * C1,
    lambda in0, in1, s0, s1, imm2: (in0 * s0 + in1 * s1).astype(np.float32),
)
OP_MAD1A = _reg(
    "ANT_MAD1A", Src0 * C0 + Src1,
    lambda in0, in1, s0, s1, imm2: (in0 * s0 + in1).astype(np.float32),
)
OP_AD2C = _reg(
    "ANT_AD2C", Src0 + Src1 + C0,
    lambda in0, in1, s0, s1, imm2: (in0 + in1 + s0).astype(np.float32),
)
OP_SQSUB = _reg(
    "ANT_SQSUB", Src1 - sq(Src0),
    lambda in0, in1, s0, s1, imm2: (in1 - in0 * in0).astype(np.float32),
)
OP_LC = _reg(
    "ANT_LC", (Src0 - Src1) * Src0,
    lambda in0, in1, s0, s1, imm2: ((in0 - in1) * in0).astype(np.float32),
)
OP_RANGE = _reg(
    "ANT_RANGE", (Src0 > Zero) & (Src0 < C0),
    lambda in0, in1, s0, s1, imm2: ((in0 > 0) & (in0 < s0)).astype(np.float32),
)
OP_SELHALF = _reg(
    "ANT_SELHALF", select(Src1 > Zero, Src0 * C0, Zero),
    lambda in0, in1, s0, s1, imm2: np.where(in1 > 0, in0 * s0, 0.0).astype(np.float32),
)
OP_SELC = _reg(
    "ANT_SELC", select(Src1 > Zero, Src0, C0),
    lambda in0, in1, s0, s1, imm2: np.where(in1 > 0, in0, s0).astype(np.float32),
)
OP_MERGE = _reg(
    "ANT_MERGE", Src0 - Src0 * (Src1 > Zero) * C0 + Src1,
    lambda in0, in1, s0, s1, imm2: (in0 - in0 * (in1 > 0) * s0 + in1).astype(np.float32),
)
OP_MSUB = _reg(
    "ANT_MSUB", Src0 * Src1,
    lambda in0, in1, s0, s1, imm2: (in0 * in1).astype(np.float32),
)


# ---------------- builder ----------------

def _build_nc():
    nc = bacc.Bacc()

    kpt = nc.declare_dram_parameter("kpt", [P, 4 * FD], F32, isOutput=False)
    cst = nc.declare_dram_parameter("cst", [P, 96], F32, isOutput=False)
    maps = nc.declare_dram_parameter("maps", [V, H, W], F32, isOutput=False)
    eidx = nc.declare_dram_parameter("eidx", [P, 2 * FD], I16, isOutput=False)
    pidx = nc.declare_dram_parameter("pidx", [P, 4 * FD], I16, isOutput=False)
    cidx = nc.declare_dram_parameter("cidx", [P, 2 * SQ * P], I16, isOutput=False)
    sidx0 = nc.declare_dram_parameter("sidx0", [P, NG * 2 * GW], I16, isOutput=False)
    sidx1 = nc.declare_dram_parameter("sidx1", [P, NG * 2 * GW], I16, isOutput=False)

    out_maps = nc.declare_dram_parameter("out_maps", [V, H, W], F32, isOutput=True)
    stats = nc.declare_dram_parameter("stats", [1, 8], F32, isOutput=True)

    with TileContext(nc) as tc:
        with (
            tc.tile_pool(name="main", bufs=1) as pool,
            tc.tile_pool(name="psum", bufs=2, space="PSUM") as psp,
        ):
            _emit(nc, tc, pool, psp, kpt, cst, maps, eidx, pidx, cidx, sidx0,
                  sidx1, out_maps, stats)
    nc.finalize()
    return nc


def _emit(nc, tc, pool, psp, kpt, cst, maps, eidx, pidx, cidx, sidx0, sidx1,
          out_maps, stats):
    cnt = itertools.count()

    def newt(shape=(P, FD), dtype=F32, name=None):
        if name is None:
            return pool.tile(list(shape), dtype, name=f"w{next(cnt)}",
                             tag="work", bufs=64)
        return pool.tile(list(shape), dtype, name=name)

    _ARITH = {mybir.AluOpType.mult, mybir.AluOpType.add,
              mybir.AluOpType.subtract, mybir.AluOpType.max,
              mybir.AluOpType.min}

    def tt(a, b, op, out=None):
        o = out if out is not None else newt()
        eng = nc.any if op in _ARITH else nc.vector
        eng.tensor_tensor(out=o[:], in0=a[:], in1=b[:], op=op)
        return o

    def ts(a, s1, op0, s2=None, op1=None, out=None):
        o = out if out is not None else newt()
        eng = nc.any if op0 in _ARITH and (op1 is None or op1 in _ARITH) \
            else nc.vector
        eng.tensor_scalar(
            out=o[:], in0=a[:], scalar1=s1, scalar2=s2,
            op0=op0, op1=op1 if op1 is not None else mybir.AluOpType.bypass)
        return o

    def stt(a, s, b, op0, op1, out=None):
        o = out if out is not None else newt()
        nc.vector.scalar_tensor_tensor(
            out=o[:], in0=a[:], scalar=s, in1=b[:], op0=op0, op1=op1)
        return o

    def cdve(op, in0, in1=None, s0=0.0, s1=0.0, out=None, shape=(P, FD)):
        o = out if out is not None else newt(shape)
        nc.vector._custom_dve(
            op, out=o[:], in0=in0[:],
            in1=in1[:] if in1 is not None else None, s0=s0, s1=s1)
        return o

    def recip(a, out=None, fast=False):
        o = out if out is not None else newt()
        if fast:
            nc.vector.reciprocal_approx_fast(out=o[:], in_=a[:])
        else:
            nc.vector.reciprocal(out=o[:], in_=a[:])
        return o

    MUL = mybir.AluOpType.mult
    ADD = mybir.AluOpType.add
    SUB = mybir.AluOpType.subtract

    # ---------- DMA in ----------
    kp = newt((P, 4 * FD), name="kp")
    nc.sync.dma_start(out=kp[:], in_=kpt[:])
    cstt = newt((P, 96), name="cstt")
    nc.sync.dma_start(out=cstt[:], in_=cst[:])
    sc = lambda j: cstt[:, j:j + 1]  # noqa: E731

    eidx_t = newt((P, 2 * FD), I16, name="eidx_t")
    nc.sync.dma_start(out=eidx_t[:], in_=eidx[:])
    pidx_t = newt((P, 4 * FD), I16, name="pidx_t")
    nc.sync.dma_start(out=pidx_t[:], in_=pidx[:])
    cidx_t = newt((P, 2 * SQ * P), I16, name="cidx_t")
    nc.sync.dma_start(out=cidx_t[:], in_=cidx[:])
    sidx0_t = newt((P, NG * 2 * GW), I16, name="sidx0_t")
    nc.sync.dma_start(out=sidx0_t[:], in_=sidx0[:])
    sidx1_t = newt((P, NG * 2 * GW), I16, name="sidx1_t")
    nc.sync.dma_start(out=sidx1_t[:], in_=sidx1[:])

    mch = []  # map chunks (v, rb) -> (128, 1024) tile
    for v in range(V):
        row = []
        for rb in range(RB):
            t = newt((P, W), name=f"mch{v}_{rb}")
            nc.sync.dma_start(out=t[:], in_=maps[v, rb * P:(rb + 1) * P, :])
            row.append(t)
        mch.append(row)

    x0 = kp[:, 0 * FD:1 * FD]
    y0 = kp[:, 1 * FD:2 * FD]
    x1 = kp[:, 2 * FD:3 * FD]
    y1 = kp[:, 3 * FD:4 * FD]

    class A:  # AP wrapper so helpers can take raw slices uniformly
        def __init__(self, ap):
            self.ap = ap

        def __getitem__(self, k):
            return self.ap

    x0, y0, x1, y1 = A(x0), A(y0), A(x1), A(y1)

    # ---------- Phase A: features, trace, M ----------
    s0 = cdve(OP_SQSUM2, x0, y0)
    s1 = cdve(OP_SQSUM2, x1, y1)
    # trace = Kt . feats + Kt6  (Kt at consts cols 70..76)
    u1 = cdve(OP_MAD2, s0, x0, sc(70), sc(71))
    u2 = cdve(OP_MAD2, y0, s1, sc(72), sc(73))
    u3 = cdve(OP_MAD2, x1, y1, sc(74), sc(75))
    u12 = tt(u1, u2, ADD)
    tr = cdve(OP_AD2C, u12, u3, sc(76))
    tri = recip(tr, out=newt(name="tri"), fast=True)
    fs0 = tt(s0, tri, MUL, out=newt(name="fs0"))
    fx0 = tt(x0, tri, MUL, out=newt(name="fx0"))
    fy0 = tt(y0, tri, MUL, out=newt(name="fy0"))
    fs1 = tt(s1, tri, MUL, out=newt(name="fs1"))
    fx1 = tt(x1, tri, MUL, out=newt(name="fx1"))
    fy1 = tt(y1, tri, MUL, out=newt(name="fy1"))

    # M entries: K row-major at consts cols 0..69 (entry e -> cols 7e..7e+6)
    Midx = [(0, 0), (0, 1), (0, 2), (0, 3), (1, 1), (1, 2), (1, 3), (2, 2),
            (2, 3), (3, 3)]
    Mt = {}
    for e in range(10):
        base = 7 * e
        v1_ = cdve(OP_MAD2, fs0, fx0, sc(base + 0), sc(base + 1))
        v2_ = cdve(OP_MAD2, fy0, fs1, sc(base + 2), sc(base + 3))
        v3_ = cdve(OP_MAD2, fx1, fy1, sc(base + 4), sc(base + 5))
        a12 = tt(v1_, v2_, ADD)
        m1 = cdve(OP_MAD1A, tri, a12, sc(base + 6))
        Mt[Midx[e]] = tt(m1, v3_, ADD, out=newt(name=f"M{e}"))

    def Mg(i, j):
        return Mt[(i, j) if i <= j else (j, i)]

    # ---------- Phase B: characteristic polynomial ----------
    t01 = tt(Mg(0, 0), Mg(1, 1), ADD)
    t23 = tt(Mg(2, 2), Mg(3, 3), ADD)
    c3 = tt(t01, t23, ADD, out=newt(name="c3"))
    # c2 = sum over pairs (Mii*Mjj - Mij^2)
    c2 = None
    for (i, j) in [(0, 1), (0, 2), (0, 3), (1, 2), (1, 3), (2, 3)]:
        pr = tt(Mg(i, i), Mg(j, j), MUL)
        term = cdve(OP_SQSUB, Mg(i, j), pr)
        c2 = term if c2 is None else tt(c2, term, ADD)
    c2n = newt(name="c2")
    nc.any.tensor_copy(out=c2n[:], in_=c2[:])
    c2 = c2n

    def det3(rows, cols, G):
        (a, b, c), (d, e, f), (g, h, i_) = [[G(r, cc) for cc in cols] for r in rows]
        # generic det3: a(ei-fh) - b(di-fg) + c(dh-eg)
        ei = tt(e, i_, MUL)
        fh = tt(f, h, MUL)
        m1 = tt(ei, fh, SUB)
        di = tt(d, i_, MUL)
        fg = tt(f, g, MUL)
        m2 = tt(di, fg, SUB)
        dh = tt(d, h, MUL)
        eg = tt(e, g, MUL)
        m3 = tt(dh, eg, SUB)
        r1 = tt(a, m1, MUL)
        r2 = tt(b, m2, MUL)
        r3 = tt(c, m3, MUL)
        s_ = tt(r1, r2, SUB)
        return tt(s_, r3, ADD)

    c1 = None
    d123 = None
    for k in range(4):
        rows = [i for i in range(4) if i != k]
        d = det3(rows, rows, Mg)
        if k == 0:
            d123 = d            # det3 over rows/cols [1,2,3] — reused by c0
        c1 = d if c1 is None else tt(c1, d, ADD)
    c1n = newt(name="c1")
    nc.any.tensor_copy(out=c1n[:], in_=c1[:])
    c1 = c1n
    c0 = None
    for j in range(4):
        cols = [c for c in range(4) if c != j]
        d = d123 if j == 0 else det3([1, 2, 3], cols, Mg)
        term = tt(Mg(0, j), d, MUL)
        if c0 is None:
            c0 = term
        elif j % 2 == 1:
            c0 = tt(c0, term, SUB)
        else:
            c0 = tt(c0, term, ADD)
    c0n = newt(name="c0")
    nc.any.tensor_copy(out=c0n[:], in_=c0[:])
    c0 = c0n

    # ---------- Phase C: Newton from 0 ----------
    rc1 = recip(c1, fast=True)
    lam = tt(c0, rc1, MUL)
    for _ in range(NEWTON_ITERS):
        s1_ = cdve(OP_LC, lam, c3)            # (lam - c3) * lam
        s2 = tt(s1_, c2, ADD)
        s3a = tt(s2, lam, MUL)
        s3 = tt(s3a, c1, SUB)
        s4a = tt(s3, lam, MUL)
        pp = tt(s4a, c0, ADD)
        d1 = cdve(OP_MAD2, lam, c3, 4.0, -3.0)
        d2a = tt(d1, lam, MUL)
        d2 = cdve(OP_MAD1A, c2, d2a, 2.0)
        d3a = tt(d2, lam, MUL)
        dp = tt(d3a, c1, SUB)
        rdp = recip(dp, fast=True)
        st = tt(pp, rdp, MUL)
        lam = tt(lam, st, SUB, out=newt(name=f"lam{next(cnt)}"))

    # ---------- Phase D: eigenvector via 4D crosses ----------
    Bd = {}
    for i in range(4):
        Bd[i] = tt(Mg(i, i), lam, SUB, out=newt(name=f"Bd{i}"))

    def Bg(i, j):
        return Bd[i] if i == j else Mg(i, j)

    # all 2x2 dets of rows (2,3): d2[(a,b)] = B2a*B3b - B2b*B3a for a<b
    d2 = {}
    for (a, b) in [(0, 1), (0, 2), (0, 3), (1, 2), (1, 3), (2, 3)]:
        m1 = tt(Bg(2, a), Bg(3, b), MUL)
        m2 = tt(Bg(2, b), Bg(3, a), MUL)
        d2[(a, b)] = tt(m1, m2, SUB, out=newt(name=f"d2_{a}{b}"))

    def cross_from_row(r0):
        # v_i = (-1)^i * det3(rows [r0,2,3], cols != i), using shared d2
        v = []
        for i in range(4):
            cols = [c for c in range(4) if c != i]
            (ca, cb, cc) = cols
            t1_ = tt(Bg(r0, ca), d2[(cb, cc)], MUL)
            t2_ = tt(Bg(r0, cb), d2[(ca, cc)], MUL)
            t3_ = tt(Bg(r0, cc), d2[(ca, cb)], MUL)
            s_ = tt(t1_, t2_, SUB)
            dd = tt(s_, t3_, ADD)
            v.append(dd)
        # apply cofactor signs: v_i *= (-1)^i — fold into downstream by
        # alternating signs; easier: negate odd entries now.
        v2_ = []
        for i, t in enumerate(v):
            if i % 2 == 1:
                v2_.append(ts(t, -1.0, MUL))
            else:
                v2_.append(t)
        return v2_

    vA = cross_from_row(1)
    vB = cross_from_row(0)
    # row-0-based cross has opposite orientation; sign irrelevant for hom.
    nA1 = cdve(OP_SQSUM2, vA[0], vA[1])
    nA2 = cdve(OP_SQSUM2, vA[2], vA[3])
    nA = tt(nA1, nA2, ADD)
    nB1 = cdve(OP_SQSUM2, vB[0], vB[1])
    nB2 = cdve(OP_SQSUM2, vB[2], vB[3])
    nB = tt(nB1, nB2, ADD)
    msk = newt((P, FD), mybir.dt.uint8, name="msk")
    nc.vector.tensor_tensor(out=msk[:], in0=nA[:], in1=nB[:],
                            op=mybir.AluOpType.is_ge)
    hom = []
    for i in range(4):
        o = newt(name=f"hom{i}")
        nc.vector.select(out=o[:], mask=msk[:], on_true=vA[i][:], on_false=vB[i][:])
        hom.append(o)

    # ---------- Phase E: z, valid, values, stats ----------
    invw = recip(hom[3])
    zs = []
    for v in range(2):
        e0, e1, e2, e3 = (77 + 4 * v, 78 + 4 * v, 79 + 4 * v, 80 + 4 * v)
        sd = ts(hom[0], sc(e0), MUL)
        sd = stt(hom[1], sc(e1), sd, MUL, ADD)
        sd = stt(hom[2], sc(e2), sd, MUL, ADD)
        zr = tt(sd, invw, MUL)
        z = ts(zr, sc(e3), ADD, out=newt(name=f"z{v}"))
        zs.append(z)
    r0 = cdve(OP_RANGE, zs[0], None, TH_MAX)
    r1 = cdve(OP_RANGE, zs[1], None, TH_MAX)
    valid = tt(r0, r1, MUL, out=newt(name="valid"))
    vals = []
    for v in range(2):
        mkz = recip(zs[v])
        vals.append(cdve(OP_SELHALF, mkz, valid, 0.5,
                         out=newt(name=f"vals{v}")))

    # near/far: per-view masked min/max, then partition reduce
    red = newt((P, 8), name="red")
    for v in range(2):
        zmin = cdve(OP_SELC, zs[v], valid, 1e30)
        zmax = cdve(OP_SELC, zs[v], valid, -1e30)
        nc.vector.tensor_reduce(out=red[:, v:v + 1], in_=zmin[:],
                                axis=mybir.AxisListType.X,
                                op=mybir.AluOpType.min, negate=True)
        nc.vector.tensor_reduce(out=red[:, 2 + v:3 + v], in_=zmax[:],
                                axis=mybir.AxisListType.X,
                                op=mybir.AluOpType.max)
    nc.vector.tensor_reduce(out=red[:, 4:5], in_=valid[:],
                            axis=mybir.AxisListType.X, op=mybir.AluOpType.max)
    nc.gpsimd.memset(red[:, 5:8], 0.0)
    # negate=True on the min-reduce writes -min, so max-allreduce works for all
    redall = newt((P, 8), name="redall")
    nc.gpsimd.partition_all_reduce(out_ap=redall[:], in_ap=red[:], channels=P,
                                   reduce_op=bass_isa.ReduceOp.max)
    nc.sync.dma_start(out=stats[:], in_=redall[0:1, :])

    # ---------- Phase F: value routing ----------
    # view 0: expand dense vals into padded-group layout E0
    E0 = newt((P, EW), name="E0")
    nc.gpsimd.local_scatter(
        out_ap=E0[:].bitcast(U16), data_ap=vals[0][:].bitcast(U16),
        idxs_ap=eidx_t[:], channels=P, num_elems=2 * EW, num_idxs=2 * FD)

    # view 1: place into squares (2 halves), PE-transpose, compact into C1
    halves = []
    for hh in range(2):
        Ph = newt((P, SQH * P), name=f"place{hh}")
        nc.gpsimd.local_scatter(
            out_ap=Ph[:].bitcast(U16), data_ap=vals[1][:].bitcast(U16),
            idxs_ap=pidx_t[:, hh * 2 * FD:(hh + 1) * 2 * FD],
            channels=P, num_elems=2 * SQH * P, num_idxs=2 * FD)
        halves.append(Ph)
    ident = newt((P, P), name="ident")
    make_identity(nc, ident[:])
    T1 = newt((P, SQ * P), name="T1")
    for s in range(SQ):
        src = halves[s // SQH]
        off = (s % SQH) * P
        ps = psp.tile([P, P], F32, name=f"ps{s}", tag="ps", bufs=2)
        nc.tensor.transpose(out=ps[:], in_=src[:, off:off + P], identity=ident[:])
        nc.scalar.copy(out=T1[:, s * P:(s + 1) * P], in_=ps[:])
    C1 = newt((P, EW), name="C1")
    nc.gpsimd.local_scatter(
        out_ap=C1[:].bitcast(U16), data_ap=T1[:].bitcast(U16),
        idxs_ap=cidx_t[:], channels=P, num_elems=2 * EW, num_idxs=2 * SQ * P)

    # ---------- Phase G: per-chunk scatter + merge + write out ----------
    EV = [E0, C1]
    SIDX = [sidx0_t, sidx1_t]
    for v in range(2):
        for rb in range(RB):
            for hh in range(CH):
                g = rb * CH + hh
                valc = pool.tile([P, CW], F32, name=f"valc{v}_{g}",
                                 tag="valc", bufs=4)
                nc.gpsimd.local_scatter(
                    out_ap=valc[:].bitcast(U16),
                    data_ap=EV[v][:, g * GW:(g + 1) * GW].bitcast(U16),
                    idxs_ap=SIDX[v][:, g * 2 * GW:(g + 1) * 2 * GW],
                    channels=P, num_elems=2 * CW, num_idxs=2 * GW)
                mslice = mch[v][rb][:, hh * CW:(hh + 1) * CW]
                nc.vector._custom_dve(OP_MERGE, out=mslice, in0=mslice,
                                      in1=valc[:], s0=0.5)
            nc.sync.dma_start(out=out_maps[v, rb * P:(rb + 1) * P, :],
                              in_=mch[v][rb][:])


_NC_CACHE = None


def _get_nc():
    global _NC_CACHE
    if _NC_CACHE is None:
        _NC_CACHE = _build_nc()
    return _NC_CACHE


# ---------------- host-side preparation ----------------

def _host_prep(mkpts0, mkpts1, mconf, mbids, intrinsics, extrinsics,
               fullres_disps):
    f32 = np.float32
    row_scale = np.array([W, H, 1.0])[None, None, :, None]
    intr = intrinsics.astype(np.float64) * row_scale
    extr_inv = np.linalg.inv(extrinsics.astype(np.float64))
    proj = np.einsum('bvij,bvjk->bvik', intr, extr_inv[:, :, :3, :])
    proj32 = proj.astype(f32)
    einv2 = extr_inv[:, :, 2, :].astype(f32)

    disps = fullres_disps.reshape(V, B, H, W)

    in_maps = []
    for b in range(B):
        sel = np.where(mbids == b)[0]
        x0 = mkpts0[sel, 0].astype(f32)
        y0 = mkpts0[sel, 1].astype(f32)
        x1 = mkpts1[sel, 0].astype(f32)
        y1 = mkpts1[sel, 1].astype(f32)
        n = len(sel)
        assert n > 0

        xi0 = np.clip(x0.astype(np.int32), 0, W - 1)
        yi0 = np.clip(y0.astype(np.int32), 0, H - 1)
        xi1 = np.clip(x1.astype(np.int32), 0, W - 1)
        yi1 = np.clip(y1.astype(np.int32), 0, H - 1)

        # last-writer-wins masks per view (in original order)
        def last_writer(yy, xx):
            flat = yy.astype(np.int64) * W + xx
            wmask = np.zeros(n, bool)
            # np.unique keeps first occurrence; reverse for last
            _, idx_rev = np.unique(flat[::-1], return_index=True)
            wmask[n - 1 - idx_rev] = True
            return wmask

        w0 = last_writer(yi0, xi0)
        w1 = last_writer(yi1, xi1)

        part = yi0 % P
        rb0, h0, c0 = yi0 // P, xi0 // CW, xi0 % CW
        rb1, h1, c1 = yi1 // P, xi1 // CW, xi1 % CW
        q = yi1 % P

        # order kps per partition by (group) for compact windows
        order = np.lexsort((rb0 * CH + h0, part))
        slot = np.zeros(n, np.int64)
        pc = np.zeros(P, np.int64)
        for k in order:
            slot[k] = pc[part[k]]
            pc[part[k]] += 1
        assert pc.max() <= FD, f"partition overflow {pc.max()} > {FD}"

        kpt = np.zeros((P, 4 * FD), f32)
        # pad slots with kp 0 (a real keypoint of this scene)
        kpt[:, 0 * FD:1 * FD] = x0[0]
        kpt[:, 1 * FD:2 * FD] = y0[0]
        kpt[:, 2 * FD:3 * FD] = x1[0]
        kpt[:, 3 * FD:4 * FD] = y1[0]
        kpt[part, 0 * FD + slot] = x0
        kpt[part, 1 * FD + slot] = y0
        kpt[part, 2 * FD + slot] = x1
        kpt[part, 3 * FD + slot] = y1

        # expand idx (view 0): dense slot -> padded group slot
        eidx = np.full((P, 2 * FD), -1, np.int16)
        sidx0 = np.full((P, NG * 2 * GW), -1, np.int16)
        gcount0 = np.zeros((P, NG), np.int64)
        for k in np.where(w0)[0]:
            p_, g_ = part[k], rb0[k] * CH + h0[k]
            j = gcount0[p_, g_]
            gcount0[p_, g_] += 1
            assert j < GW, "group overflow v0"
            tgt = g_ * GW + j
            eidx[p_, 2 * slot[k]] = 2 * tgt
            eidx[p_, 2 * slot[k] + 1] = 2 * tgt + 1
            sidx0[p_, g_ * 2 * GW + 2 * j] = 2 * c0[k]
            sidx0[p_, g_ * 2 * GW + 2 * j + 1] = 2 * c0[k] + 1

        # view 1 routing: squares placement, compact, chunk idx
        pidx = np.full((P, 4 * FD), -1, np.int16)
        cidxA = np.full((P, 2 * SQ * P), -1, np.int16)
        sidx1 = np.full((P, NG * 2 * GW), -1, np.int16)
        sqcount = np.zeros((P, P), np.int64)
        gcount1 = np.zeros((P, NG), np.int64)
        for k in np.where(w1)[0]:
            p_, q_ = part[k], q[k]
            s_ = sqcount[p_, q_]
            sqcount[p_, q_] += 1
            assert s_ < SQ, "square overflow"
            hh = 0 if s_ < SQH else 1
            s_in = s_ - hh * SQH
            tgt = s_in * P + q_
            pidx[p_, hh * 2 * FD + 2 * slot[k]] = 2 * tgt
            pidx[p_, hh * 2 * FD + 2 * slot[k] + 1] = 2 * tgt + 1
            # after transpose: at partition q_, f32 col s_*P + p_
            g_ = rb1[k] * CH + h1[k]
            j = gcount1[q_, g_]
            gcount1[q_, g_] += 1
            assert j < GW, "group overflow v1"
            tpos = s_ * P + p_
            ctgt = g_ * GW + j
            cidxA[q_, 2 * tpos] = 2 * ctgt
            cidxA[q_, 2 * tpos + 1] = 2 * ctgt + 1
            sidx1[q_, g_ * 2 * GW + 2 * j] = 2 * c1[k]
            sidx1[q_, g_ * 2 * GW + 2 * j + 1] = 2 * c1[k] + 1

        # consts row
        K = np.zeros((10, 7), f32)
        Midx = [(0, 0), (0, 1), (0, 2), (0, 3), (1, 1), (1, 2), (1, 3),
                (2, 2), (2, 3), (3, 3)]
        for e, (i, j) in enumerate(Midx):
            for v in range(2):
                a = proj32[b, v, 2]
                bb = proj32[b, v, 0]
                cc = proj32[b, v, 1]
                base = 0 if v == 0 else 3
                K[e, base + 0] += a[i] * a[j]
                K[e, base + 1] += -(a[i] * bb[j] + a[j] * bb[i])
                K[e, base + 2] += -(a[i] * cc[j] + a[j] * cc[i])
                K[e, 6] += bb[i] * bb[j] + cc[i] * cc[j]
        Kt = K[[0, 4, 7, 9]].sum(axis=0)
        row = np.zeros(96, f32)
        row[0:70] = K.reshape(-1)
        row[70:77] = Kt
        row[77:81] = einv2[b, 0]
        row[81:85] = einv2[b, 1]
        cstrow = np.tile(row[None, :], (P, 1))

        in_maps.append({
            "kpt": kpt,
            "cst": cstrow,
            "maps": np.ascontiguousarray(disps[:, b]),
            "eidx": eidx,
            "pidx": pidx,
            "cidx": cidxA,
            "sidx0": sidx0,
            "sidx1": sidx1,
        })
    return in_maps


def kernel(mkpts0, mkpts1, mconf, mbids, intrinsics, extrinsics,
           fullres_disps):
    global LAST_RESULTS
    import os
    from concourse.bass_utils import run_bass_kernel_spmd

    nc = _get_nc()
    in_maps = _host_prep(np.asarray(mkpts0), np.asarray(mkpts1),
                         np.asarray(mconf), np.asarray(mbids),
                         np.asarray(intrinsics), np.asarray(extrinsics),
                         np.asarray(fullres_disps))

    res = run_bass_kernel_spmd(nc, in_maps, core_ids=list(range(8)),
                               trace=bool(os.environ.get("KERNEL_TRACE")))
    LAST_RESULTS = res

    fused = np.zeros((V, B, H, W), np.float32)
    near = np.zeros((B, V), np.float32)
    far = np.zeros((B, V), np.float32)
    flag = np.zeros((B, V), np.float32)
    for b in range(B):
        r = res.results[b]
        fused[:, b] = r["out_maps"]
        st = r["stats"][0]
        has = st[4] > 0
        for v in range(2):
            near[b, v] = -st[v] if has else 0.0
            far[b, v] = st[2 + v] if has else 500.0
            flag[b, v] = 1.0 if has else 0.0
    return (fused.reshape(V * B, 1, H, W), near, far, flag)
